# revision 81
# baseline (speedup 1.0000x reference)
"""Causal multi-head attention (B=2, S=2048, D=1024, H=16) on 8 TRN2 NeuronCores.

Sharding: batch*heads across cores. Core c handles batch c//4 and the 4 heads
g*4..g*4+3 where g = c%4. Weights are sliced per core (Megatron-style column
split of Wq/Wk/Wv, row split of Wo); each core produces a partial projected
output [D, S] (transposed) and the host sums the 4 partials per batch.

Everything on-chip is kept transposed ([feature, seq]) so no transposes are
ever needed:
  qT/kT = wq/wk^T @ xT            (PE, bf16, contraction over D)
  v     = x @ Wv^T                (bf16; s on partitions, + ones col appended)
  sT    = k @ qT  [s_k=128, s_q]  (PE, contraction over dh=64, 2 heads packed
                                   via base-partition 0/64 row groups)
  eT    = exp(sT/8) -> bf16, then an on-chip 0/1 mask (built once with
          affine_select) zeroes the 128-wide causal band (GPSIMD)
  av    = v_aug^T @ eT -> [65, s_q]  row 64 = softmax denominator
  outT  = av[0:64] * (1/denom broadcast down 64 partitions via ones matmul)
  partialT = wo^T-chunks @ outT   (PE, fp32r, contraction over 256 head dims)

x/Wq/Wk/Wv stream in as bf16 (halves input DMA; the kernel head is DMA-feed
bound) and partials stream out as bf16 (host sums in float64); measured
relative error 3.6e-3 against the fp32 reference.

Scheduling: engines execute their instruction streams in emission order, so
the kernel is emitted as ONE interleaved stream. Attention chunks (latency-
bound on the PE->ACT->GPSIMD->PE chain) are diluted with fine-grained filler
units (2-4 matmuls each): QKV work for later tiles, deferred Wo work for
earlier tiles, and deferred softmax-denominator broadcasts (fins). A per-phase
emission-time ledger of ACT-vs-PE nanoseconds pumps fillers exactly when the
scalar engine (exp) would fall behind; AV matmuls trail their scores by
AV_LAG chunks. Wo groups are held back and consumed during the last tile's
attention (which is otherwise ACT-bound). The chunk-major prologue advances
all four k/q accumulators per arriving x-block pair (their PSUM lives in the
scores pool, idle until then), and the tail runs all eight Wo u=0 matmuls
across 8 borrowed PSUM slots while the final denominator chain drains, with
paired bf16 output DMAs (HWDGE issue cost dominates transfers).
"""

from collections import deque

import numpy as np

import concourse.bass as bass
import concourse.mybir as mybir
import concourse.tile as tile
from concourse import bacc
from concourse.bass_utils import run_bass_kernel_spmd

B = 2
S = 2048
D = 1024
H = 16
DH = 64
N_CORES = 8
HG = H // 4  # 4 heads per core
GM = 4 * DH  # 256 head dims per core
FP32 = mybir.dt.float32
FP32R = mybir.dt.float32r

S_TILE = 512  # q-tile width (PSUM bank)
N_ST = S // S_TILE  # 4
KC = 128  # k-chunk (partition dim of scoresT)
N_KC = S // KC  # 16
N_DC = D // 128  # 8 d-chunks
AV_LAG = 4  # chunks between scores and their AV matmuls (hides exp+mask latency)


def build_program():
    nc = bacc.Bacc("TRN2", target_bir_lowering=False, debug=False)

    # x and the QKV projection weights stream in as bf16: halves the input DMA
    # (the kernel head is DMA-feed-bound) at a ~5e-3 relative-error cost,
    # comfortably inside the 2e-2 budget. Wo and the softmax math stay fp32r/
    # fp32. Partial outputs are bf16 (host sums 4 partials per batch in
    # float64), halving the output DMA and the end-of-kernel drain.
    BF16 = mybir.dt.bfloat16
    xT = nc.dram_tensor("xT", [D, S], BF16, kind="ExternalInput")
    wq = nc.dram_tensor("wq", [D, GM], BF16, kind="ExternalInput")
    wk = nc.dram_tensor("wk", [D, GM], BF16, kind="ExternalInput")
    wv = nc.dram_tensor("wv", [D, GM], BF16, kind="ExternalInput")
    wo = nc.dram_tensor("wo", [GM, D], FP32R, kind="ExternalInput")
    outT = nc.dram_tensor("outT", [D, S], BF16, kind="ExternalOutput")

    with tile.TileContext(nc) as tc:
        with (
            tc.tile_pool(name="persist", bufs=1) as persist,
            tc.tile_pool(name="xb", bufs=8) as xb_pool,
            tc.tile_pool(name="exp", bufs=10) as exp_pool,
            tc.tile_pool(name="small", bufs=4) as small_pool,
            tc.tile_pool(name="outsb", bufs=4) as out_pool,
            tc.tile_pool(name="mm", bufs=2, space="PSUM") as mm_pool,
            tc.tile_pool(name="scores", bufs=2, space="PSUM") as sc_pool,
            tc.tile_pool(name="av", bufs=2, space="PSUM") as av_pool,
        ):
            # ---- persistent SBUF tensors ----
            wo_sb = persist.tile([128, 2, D], FP32R, tag="wo")
            ones_col = persist.tile([128, 1], FP32, tag="ones")
            ones_row = persist.tile([1, DH], FP32R, tag="onesr")
            nc.vector.memset(ones_col[:, :], 1.0)
            nc.vector.tensor_copy(
                ones_row[:, :], ones_col[0:1, 0:1].broadcast_to((1, DH))
            )
            # causal mask patterns, generated on-chip (no DMA):
            # mask4[p, j, q] = 1.0 iff 128*j + p <= q
            mask4 = persist.tile([128, 4, S_TILE], FP32, tag="mask4")
            nc.gpsimd.memset(mask4[:, :, :], 1.0)
            for j in range(4):
                nc.gpsimd.affine_select(
                    mask4[:, j, :],
                    mask4[:, j, :],
                    pattern=[[1, S_TILE]],
                    compare_op=mybir.AluOpType.is_ge,
                    fill=0.0,
                    base=-128 * j,
                    channel_multiplier=-1,
                )
            w_sb = {}
            for name in ("q", "k", "v"):
                w_sb[name] = persist.tile(
                    [128, N_DC, GM], BF16, tag=f"w{name}", name=f"w{name}sb"
                )

            qT = {}  # (u, t) -> [128, 512]   2 heads stacked (rows 0-63 / 64-127)
            kT = {}
            vt = {}  # c16 -> [128, HG, 65]   v chunk with ones col per head
            oT = {}  # (u, t) -> [128, 512]
            for t in range(N_ST):
                for u in range(2):
                    qT[(u, t)] = persist.tile(
                        [128, S_TILE], BF16, tag=f"qT{u}{t}", name=f"qT{u}{t}"
                    )
                    kT[(u, t)] = persist.tile(
                        [128, S_TILE], BF16, tag=f"kT{u}{t}", name=f"kT{u}{t}"
                    )
                    oT[(u, t)] = persist.tile(
                        [128, S_TILE], FP32R, tag=f"oT{u}{t}", name=f"oT{u}{t}"
                    )
            for c16 in range(N_KC):
                vt[c16] = persist.tile(
                    [128, HG, DH + 1], BF16, tag=f"v{c16}", name=f"v{c16}"
                )

            xb = {}

            def load_xb(t, c0):
                # two d-chunks per DMA: halves the HWDGE issue cost (625ns
                # per DMA vs 364ns bf16 transfer, so issue rate dominates)
                blk = xb_pool.tile(
                    [128, 2, S_TILE], BF16, tag="xb", name=f"xb{c0}_{t}"
                )
                nc.sync.dma_start(
                    blk[:, :, :],
                    xT[
                        c0 * 128 : (c0 + 2) * 128,
                        t * S_TILE : (t + 1) * S_TILE,
                    ].rearrange("(i p) m -> p i m", i=2),
                    )
                xb[(c0, t)] = blk[:, 0, :]
                xb[(c0 + 1, t)] = blk[:, 1, :]

            # ---- DMA stream order ----
            # tile-0 inputs first (weights in halves interleaved with x blocks
            # so the first k/q matmuls start ~3.5us in), then x tiles 1..3
            # stream ahead of their QKV filler units, then wo (needed late).
            wk_r = wk.rearrange("(c p) m -> p c m", p=128)
            wq_r = wq.rearrange("(c p) m -> p c m", p=128)
            wv_r = wv.rearrange("(c p) m -> p c m", p=128)
            # paced for the chunk-major prologue: per 2-chunk period, the k and
            # q weight chunks land just before the x block pair that uses them
            for p in range(4):
                nc.sync.dma_start(
                    w_sb["k"][:, 2 * p : 2 * p + 2, :], wk_r[:, 2 * p : 2 * p + 2, :]
                )
                if p > 0:
                    nc.sync.dma_start(
                        w_sb["q"][:, 2 * p : 2 * p + 2, :],
                        wq_r[:, 2 * p : 2 * p + 2, :],
                    )
                load_xb(0, 2 * p)
                if p == 0:
                    nc.sync.dma_start(
                        w_sb["q"][:, 0:2, :], wq_r[:, 0:2, :]
                    )
            nc.sync.dma_start(w_sb["v"][:, 0:4, :], wv_r[:, 0:4, :])
            nc.sync.dma_start(w_sb["v"][:, 4:8, :], wv_r[:, 4:8, :])
            for c0 in range(0, N_DC, 2):
                load_xb(1, c0)
            nc.sync.dma_start(wo_sb[:, :, :], wo.rearrange("(u p) d -> p u d", p=128))
            for t in range(2, N_ST):
                for c0 in range(0, N_DC, 2):
                    load_xb(t, c0)

            # ---- emission thunks ----
            # Group PSUM tiles are created lazily by the first unit of each
            # group (cell dict) so mm_pool's buffer rotation follows actual
            # use order, not enqueue order.
            def qk_matmuls(name, u, t, cell, c0, c1):
                if c0 == 0:
                    cell["ps"] = mm_pool.tile(
                        [128, S_TILE], FP32, tag="mm", name=f"ps{name}{u}{t}"
                    )
                ps = cell["ps"]
                for c in range(c0, c1):
                    nc.tensor.matmul(
                        ps[:, :],
                        lhsT=w_sb[name][:, c, u * 128 : (u + 1) * 128],
                        rhs=xb[(c, t)][:, :],
                        start=(c == 0),
                        stop=(c == N_DC - 1),
                    )
                if c1 == N_DC:
                    dst = kT if name == "k" else qT
                    nc.vector.tensor_copy(dst[(u, t)][:, :], ps[:, :])

            def v_matmuls(t, s4, cell, c0, c1):
                c16 = 4 * t + s4
                if c0 == 0:
                    cell["ps"] = mm_pool.tile(
                        [128, GM], FP32, tag="mm", name=f"psv{c16}"
                    )
                ps = cell["ps"]
                for c in range(c0, c1):
                    nc.tensor.matmul(
                        ps[:, :],
                        lhsT=xb[(c, t)][:, s4 * 128 : (s4 + 1) * 128],
                        rhs=w_sb["v"][:, c, :],
                        start=(c == 0),
                        stop=(c == N_DC - 1),
                    )
                if c1 == N_DC:
                    nc.vector.tensor_copy(
                        vt[c16][:, :, 0:DH], ps.rearrange("p (h d) -> p h d", h=HG)
                    )
                    nc.gpsimd.tensor_copy(
                        vt[c16][:, :, DH : DH + 1],
                        ones_col[:, 0:1].broadcast_to((128, HG, 1)),
                    )

            def emit_wo_group(t, dc, on_act=False):
                po = mm_pool.tile([128, S_TILE], FP32, tag="mm", name=f"po{t}{dc}")
                for u in range(2):
                    nc.tensor.matmul(
                        po[:, :],
                        lhsT=wo_sb[:, u, dc * 128 : (dc + 1) * 128],
                        rhs=oT[(u, t)][:, :],
                        start=(u == 0),
                        stop=(u == 1),
                    )
                ob = out_pool.tile([128, S_TILE], BF16, tag="ob")
                if on_act:  # tail: split the copy across ACT+DVE so the PSUM
                    # bank frees at PE pace, not copy pace
                    nc.scalar.copy(ob[:, 0 : S_TILE // 2], po[:, 0 : S_TILE // 2])
                    nc.vector.tensor_copy(ob[:, S_TILE // 2 :], po[:, S_TILE // 2 :])
                else:
                    nc.vector.tensor_copy(ob[:, :], po[:, :])
                nc.sync.dma_start(
                    outT[dc * 128 : (dc + 1) * 128, t * S_TILE : (t + 1) * S_TILE],
                    ob[:, :],
                )

            # filler queue: (tile_tag, cost_cycles, thunk). Attention chunks pump
            # filler units between chunks to keep the PE dense while exp/mask
            # latency elapses. QKV units are tagged with their tile (flushed
            # before that tile's attention); Wo units are tagged 99 (pump/drain
            # only -- they are enqueued once their oT inputs exist).
            fillers = deque()

            def enqueue_qkv_units(t):
                for name in ("k", "q"):
                    for u in range(2):
                        cell = {}
                        for c0 in range(0, N_DC, 2):
                            fillers.append(
                                (
                                    t,
                                    2 * S_TILE,
                                    lambda name=name, u=u, t=t, cell=cell, c0=c0: qk_matmuls(
                                        name, u, t, cell, c0, c0 + 2
                                    ),
                                )
                            )
                for s4 in range(4):
                    cell = {}
                    for c0 in range(0, N_DC, 4):
                        fillers.append(
                            (
                                t,
                                4 * GM,
                                lambda t=t, s4=s4, cell=cell, c0=c0: v_matmuls(
                                    t, s4, cell, c0, c0 + 4
                                ),
                            )
                        )

            # --- emission-time ACT-vs-PE ledger (reset per attention phase).
            # exp work accumulates act_ns; attention matmuls + pumped fillers
            # accumulate pe_ns. Pump fillers whenever ACT is ahead, so the PE
            # stream is diluted exactly where the scalar engine needs time.
            PE_CY = 1.0 / 2.4  # ns per cycle at peak
            SLACK = 900.0  # exp-pipeline fill depth: don't pump before ACT is
            # genuinely ahead of the PE stream in wall-clock terms
            ledger = {"pe": 0.0, "act": -SLACK}

            fins = deque()  # deferred den-broadcast units: popped by pumps at
            # least two chunks into the next phase, when their reciprocal
            # (ACT den copy -> DVE recip) has drained
            phase_chunk = {"c": 0}

            def pump_ledger(max_tag, extra=0.0):
                ledger["act"] += extra
                while ledger["pe"] < ledger["act"] and (fins or fillers):
                    if fins and phase_chunk["c"] >= 2:
                        cost, thunk = fins.popleft()
                    elif fillers:
                        tag, cost, thunk = fillers[0]
                        if tag != 99 and tag > max_tag:
                            return
                        fillers.popleft()
                    else:
                        return
                    thunk()
                    ledger["pe"] += cost * PE_CY

            def flush_tile(t):
                # 99 = wo units (pump/drain only, reserved as late filler)
                while fillers and fillers[0][0] <= t:
                    _, _, thunk = fillers.popleft()
                    thunk()

            # prologue, chunk-major: all four k/q accumulation groups advance
            # per arriving x block (their PSUM lives in sc_pool, idle during
            # the prologue), so the PE has ~850ns of work per ~1.5us DMA chunk
            # instead of ~430ns. v groups become tile-0 fillers.
            sc0 = sc_pool.tile([128, 2 * S_TILE], FP32, tag="sc", name="pro_sc0")
            sc1 = sc_pool.tile([128, 2 * S_TILE], FP32, tag="sc", name="pro_sc1")
            pro = {("k", 0): sc0[:, 0:S_TILE], ("q", 0): sc0[:, S_TILE:],
                   ("k", 1): sc1[:, 0:S_TILE], ("q", 1): sc1[:, S_TILE:]}
            for s4 in range(4):
                cell = {}
                for c0 in range(0, N_DC, 4):
                    fillers.append(
                        (
                            0,
                            4 * GM,
                            lambda s4=s4, cell=cell, c0=c0: v_matmuls(
                                0, s4, cell, c0, c0 + 4
                            ),
                        )
                    )
            for c in range(N_DC):
                for name in ("k", "q"):
                    for u in range(2):
                        nc.tensor.matmul(
                            pro[(name, u)],
                            lhsT=w_sb[name][:, c, u * 128 : (u + 1) * 128],
                            rhs=xb[(c, 0)][:, :],
                            start=(c == 0),
                            stop=(c == N_DC - 1),
                        )
            for u in range(2):  # ACT is idle during the prologue; keep DVE free
                nc.scalar.copy(kT[(u, 0)][:, :], pro[("k", u)])
                nc.scalar.copy(qT[(u, 0)][:, :], pro[("q", u)])
            for t in range(1, N_ST):
                enqueue_qkv_units(t)

            for t in range(N_ST):
                nch = 4 * t + 4
                for hp in range(2):
                    u = hp
                    if t > 0 or hp > 0:
                        flush_tile(t)  # qkv(<=t) must be emitted
                    ledger["pe"] = 0.0
                    ledger["act"] = -SLACK
                    phase_chunk["c"] = 0
                    avs = [
                        av_pool.tile(
                            [DH + 1, S_TILE], FP32, tag="av", name=f"av{t}{hp}{i}"
                        )
                        for i in range(2)
                    ]
                    pending_avs = deque()  # AV trails scores by AV_LAG chunks
                    for c in range(nch):
                        # Diagonal chunks only touch q columns >= 128j
                        # (causal): scores / exp / AV skip the masked prefix.
                        j = c - 4 * t
                        q0 = 128 * j if j >= 0 else 0
                        w = S_TILE - q0
                        sc = sc_pool.tile([128, 2 * S_TILE], FP32, tag="sc")
                        for i in range(2):  # head parity: rows 0-63 / 64-127
                            bp = 64 * i
                            nc.tensor.matmul(
                                sc[:, i * S_TILE + q0 : (i + 1) * S_TILE],
                                lhsT=kT[(u, c // 4)][
                                    bp : bp + DH, (c % 4) * 128 : (c % 4 + 1) * 128
                                ],
                                rhs=qT[(u, t)][bp : bp + DH, q0:],
                                start=True,
                                stop=True,
                            )
                        ledger["pe"] += 2 * w * PE_CY
                        ex = exp_pool.tile([128, 2 * S_TILE], BF16, tag="ex")
                        exv = ex.rearrange("p (i n) -> p i n", i=2)[:, :, q0:]
                        scv = sc.rearrange("p (i n) -> p i n", i=2)[:, :, q0:]
                        nc.scalar.activation(
                            exv, scv, mybir.ActivationFunctionType.Exp, scale=0.125
                        )
                        ledger["act"] += 2 * w * 0.833 + 242
                        if j >= 0:
                            # zero the causal triangle (mask is 0/1, exact).
                            # Only the band [q0, 128j+128) needs masking; one
                            # op covers both head slots via a broadcast mask.
                            bhi = 128 * j + 128
                            exb = ex.rearrange("p (i n) -> p i n", i=2)[
                                :, :, q0:bhi
                            ]
                            nc.gpsimd.tensor_mul(
                                exb,
                                exb,
                                mask4[:, j : j + 1, q0:bhi].broadcast_to(
                                    (128, 2, bhi - q0)
                                ),
                            )

                        def emit_av(cc, exx, qq0):
                            for i in range(2):
                                nc.tensor.matmul(
                                    avs[i][:, qq0:],
                                    lhsT=vt[cc][:, 2 * hp + i, :],
                                    rhs=exx[:, i * S_TILE + qq0 : (i + 1) * S_TILE],
                                    start=(cc == 0),
                                    stop=(cc == nch - 1),
                                )
                            ledger["pe"] += 2 * (S_TILE - qq0) * PE_CY

                        pending_avs.append((c, ex, q0))
                        phase_chunk["c"] = c
                        pump_ledger(t + 1)
                        if len(pending_avs) > AV_LAG:
                            emit_av(*pending_avs.popleft())
                    if t == 0 and hp == 0:
                        flush_tile(0)  # v(t0) needed by the AV drain below
                    while pending_avs:
                        emit_av(*pending_avs.popleft())
                        # the exp tail is still draining on ACT in wall-clock
                        # terms; keep filler between the drain AVs
                        pump_ledger(t + 1, extra=800)
                    # softmax denominators: rec_i = 1/den_i broadcast down the
                    # 64 partitions via a K=1 ones matmul (per head slot).
                    # The DVE chain (den copy -> reciprocal -> fp32r cast) has
                    # ~2us latency before the bc matmul can run: pump filler
                    # through it via the ledger's extra term.
                    # softmax denominators: only den copy (ACT) + reciprocal
                    # (DVE) run here; the bc broadcast matmul + oT muls are
                    # deferred as a front-of-queue filler unit so the PE never
                    # parks on the reciprocal chain -- the next phase pumps the
                    # unit 1-2 chunks in, when the reciprocal is long done.
                    recs = []
                    for i in range(2):
                        den = small_pool.tile([1, S_TILE], FP32, tag="den")
                        nc.scalar.copy(den[:, :], avs[i][DH : DH + 1, :])
                        rec32 = small_pool.tile([1, S_TILE], FP32, tag="rec32")
                        nc.vector.reciprocal_approx_fast(rec32[:, :], den[:, :])
                        recs.append(rec32)

                    def emit_fin(avs=avs, recs=recs, u=u, t=t, hp=hp):
                        for i in range(2):
                            rec = small_pool.tile([1, S_TILE], FP32R, tag="rec")
                            nc.vector.tensor_copy(rec[:, :], recs[i][:, :])
                            bc = mm_pool.tile(
                                [DH, S_TILE], FP32, tag="mm", name=f"bc{t}{hp}{i}"
                            )
                            nc.tensor.matmul(
                                bc[:, :],
                                lhsT=ones_row[:, :],
                                rhs=rec[:, :],
                                start=True,
                                stop=True,
                            )
                            bc_sb = small_pool.tile([DH, S_TILE], FP32, tag="bcsb")
                            nc.vector.tensor_copy(bc_sb[:, :], bc[:, :])
                            nc.vector.tensor_mul(
                                oT[(u, t)][64 * i : 64 * i + DH, :],
                                avs[i][0:DH, :],
                                bc_sb[:, :],
                            )

                    if t == N_ST - 1 and hp == 1:
                        emit_fin()  # no later phase to pump it
                    else:
                        fins.append((2 * S_TILE, emit_fin))
                # Wo for this tile becomes filler work for later attention
                # (the last tile's Wo is the kernel tail, emitted below)
                if t < N_ST - 1:
                    for dc in range(N_DC):
                        fillers.append(
                            (99, 2 * S_TILE, lambda t=t, dc=dc: emit_wo_group(t, dc))
                        )
            while fillers:
                fillers.popleft()[2]()
            while fins:
                fins.popleft()[1]()
            # kernel tail: the last tile's 8 Wo groups across 8 borrowed PSUM
            # slots (scores/av pools are idle now). All u=0 matmuls go first --
            # they only need oT(0,t3), which is ready long before the final
            # den-broadcast chain produces oT(1,t3) -- so the PE crunches
            # 3.4us of u0 work while that chain drains. Then the u=1 matmuls
            # close each accumulation, with ob copies split ACT/DVE.
            tail_a = sc_pool.tile([128, 2 * S_TILE], FP32, tag="sc", name="tail_a")
            tail_b = sc_pool.tile([128, 2 * S_TILE], FP32, tag="sc", name="tail_b")
            slots = [
                tail_a[:, 0:S_TILE],
                tail_a[:, S_TILE:],
                tail_b[:, 0:S_TILE],
                tail_b[:, S_TILE:],
                mm_pool.tile([128, S_TILE], FP32, tag="mm", name="tail_m0")[:, :],
                mm_pool.tile([128, S_TILE], FP32, tag="mm", name="tail_m1")[:, :],
                av_pool.tile([128, S_TILE], FP32, tag="av", name="tail_v0")[:, :],
                av_pool.tile([128, S_TILE], FP32, tag="av", name="tail_v1")[:, :],
            ]
            tl = N_ST - 1
            for u in range(2):
                for dc in range(N_DC):
                    nc.tensor.matmul(
                        slots[dc],
                        lhsT=wo_sb[:, u, dc * 128 : (dc + 1) * 128],
                        rhs=oT[(u, tl)][:, :],
                        start=(u == 0),
                        stop=(u == 1),
                    )
                    if u == 1 and dc % 2 == 1:
                        # same-tile writes serialize, so both halves of a pair
                        # go to ONE engine; pairs alternate ACT/DVE so two
                        # pairs' copies run concurrently. One DMA per pair
                        # (HWDGE issue cost dominates the bf16 transfer).
                        ob = out_pool.tile(
                            [128, 2, S_TILE], BF16, tag="ob", name=f"ob{dc}"
                        )
                        nc.scalar.copy(ob[:, 0, :], slots[dc - 1])
                        nc.vector.tensor_copy(ob[:, 1, :], slots[dc])
                        nc.sync.dma_start(
                            outT[
                                (dc - 1) * 128 : (dc + 1) * 128,
                                tl * S_TILE : (tl + 1) * S_TILE,
                            ].rearrange("(i p) m -> p i m", i=2),
                            ob[:, :, :],
                        )
    nc.compile()
    return nc


_NC_CACHE = None


def _get_program():
    global _NC_CACHE
    if _NC_CACHE is None:
        _NC_CACHE = build_program()
    return _NC_CACHE


def _make_in_maps(x, Wq, Wk, Wv, Wo):
    import ml_dtypes

    bf16 = ml_dtypes.bfloat16
    xTs = [np.ascontiguousarray(x[b].T).astype(bf16) for b in range(B)]
    in_maps = []
    for core in range(N_CORES):
        b, g = divmod(core, HG)
        r0, r1 = g * GM, (g + 1) * GM
        in_maps.append(
            {
                "xT": xTs[b],
                "wq": np.ascontiguousarray(Wq[r0:r1, :].T).astype(bf16),
                "wk": np.ascontiguousarray(Wk[r0:r1, :].T).astype(bf16),
                "wv": np.ascontiguousarray(Wv[r0:r1, :].T).astype(bf16),
                "wo": np.ascontiguousarray(Wo[:, r0:r1].T),
            }
        )
    return in_maps


def kernel(x, Wq, Wk, Wv, Wo, **_unused):
    x = np.asarray(x, dtype=np.float32)
    Wq = np.asarray(Wq, dtype=np.float32)
    Wk = np.asarray(Wk, dtype=np.float32)
    Wv = np.asarray(Wv, dtype=np.float32)
    Wo = np.asarray(Wo, dtype=np.float32)

    nc = _get_program()
    in_maps = _make_in_maps(x, Wq, Wk, Wv, Wo)
    res = run_bass_kernel_spmd(nc, in_maps, core_ids=list(range(N_CORES)))
    out = np.zeros((B, S, D), dtype=np.float64)
    for core in range(N_CORES):
        b = core // HG
        out[b] += res.results[core]["outT"].T.astype(np.float64)
    return out.astype(np.float32)


# revision 87
# speedup vs baseline: 1.0035x; 1.0035x over previous
"""Causal multi-head attention (B=2, S=2048, D=1024, H=16) on 8 TRN2 NeuronCores.

Sharding: batch*heads across cores. Core c handles batch c//4 and the 4 heads
g*4..g*4+3 where g = c%4. Weights are sliced per core (Megatron-style column
split of Wq/Wk/Wv, row split of Wo); each core produces a partial projected
output [D, S] (transposed) and the host sums the 4 partials per batch.

Everything on-chip is kept transposed ([feature, seq]) so no transposes are
ever needed:
  qT/kT = wq/wk^T @ xT            (PE, bf16, contraction over D)
  v     = x @ Wv^T                (bf16; s on partitions, + ones col appended)
  sT    = k @ qT  [s_k=128, s_q]  (PE, contraction over dh=64, 2 heads packed
                                   via base-partition 0/64 row groups)
  eT    = exp(sT/8) -> bf16, then an on-chip 0/1 mask (built once with
          affine_select) zeroes the 128-wide causal band (GPSIMD)
  av    = v_aug^T @ eT -> [65, s_q]  row 64 = softmax denominator
  outT  = av[0:64] * (1/denom broadcast down 64 partitions via ones matmul)
  partialT = wo^T-chunks @ outT   (PE, fp32r, contraction over 256 head dims)

x/Wq/Wk/Wv stream in as bf16 (halves input DMA; the kernel head is DMA-feed
bound) and partials stream out as bf16 (host sums in float64); measured
relative error 3.6e-3 against the fp32 reference.

Scheduling: engines execute their instruction streams in emission order, so
the kernel is emitted as ONE interleaved stream. Attention chunks (latency-
bound on the PE->ACT->GPSIMD->PE chain) are diluted with fine-grained filler
units (2-4 matmuls each): QKV work for later tiles, deferred Wo work for
earlier tiles, and deferred softmax-denominator broadcasts (fins). A per-phase
emission-time ledger of ACT-vs-PE nanoseconds pumps fillers exactly when the
scalar engine (exp) would fall behind; AV matmuls trail their scores by
AV_LAG chunks. Wo groups are held back and consumed during the last tile's
attention (which is otherwise ACT-bound). The chunk-major prologue advances
all four k/q accumulators per arriving x-block pair (their PSUM lives in the
scores pool, idle until then), and the tail runs all eight Wo u=0 matmuls
across 8 borrowed PSUM slots while the final denominator chain drains, with
paired bf16 output DMAs (HWDGE issue cost dominates transfers).
"""

from collections import deque

import numpy as np

import concourse.bass as bass
import concourse.mybir as mybir
import concourse.tile as tile
from concourse import bacc
from concourse.bass_utils import run_bass_kernel_spmd

B = 2
S = 2048
D = 1024
H = 16
DH = 64
N_CORES = 8
HG = H // 4  # 4 heads per core
GM = 4 * DH  # 256 head dims per core
FP32 = mybir.dt.float32
FP32R = mybir.dt.float32r

S_TILE = 512  # q-tile width (PSUM bank)
N_ST = S // S_TILE  # 4
KC = 128  # k-chunk (partition dim of scoresT)
N_KC = S // KC  # 16
N_DC = D // 128  # 8 d-chunks
AV_LAG = 4  # chunks between scores and their AV matmuls (hides exp+mask latency)


def build_program():
    nc = bacc.Bacc("TRN2", target_bir_lowering=False, debug=False)

    # x and the QKV projection weights stream in as bf16: halves the input DMA
    # (the kernel head is DMA-feed-bound) at a ~5e-3 relative-error cost,
    # comfortably inside the 2e-2 budget. Wo and the softmax math stay fp32r/
    # fp32. Partial outputs are bf16 (host sums 4 partials per batch in
    # float64), halving the output DMA and the end-of-kernel drain.
    BF16 = mybir.dt.bfloat16
    xT = nc.dram_tensor("xT", [D, S], BF16, kind="ExternalInput")
    wq = nc.dram_tensor("wq", [D, GM], BF16, kind="ExternalInput")
    wk = nc.dram_tensor("wk", [D, GM], BF16, kind="ExternalInput")
    wv = nc.dram_tensor("wv", [D, GM], BF16, kind="ExternalInput")
    wo = nc.dram_tensor("wo", [GM, D], FP32R, kind="ExternalInput")
    outT = nc.dram_tensor("outT", [D, S], BF16, kind="ExternalOutput")

    with tile.TileContext(nc) as tc:
        with (
            tc.tile_pool(name="persist", bufs=1) as persist,
            tc.tile_pool(name="xb", bufs=8) as xb_pool,
            tc.tile_pool(name="exp", bufs=10) as exp_pool,
            tc.tile_pool(name="small", bufs=4) as small_pool,
            tc.tile_pool(name="outsb", bufs=4) as out_pool,
            tc.tile_pool(name="mm", bufs=2, space="PSUM") as mm_pool,
            tc.tile_pool(name="scores", bufs=2, space="PSUM") as sc_pool,
            tc.tile_pool(name="av", bufs=2, space="PSUM") as av_pool,
        ):
            # ---- persistent SBUF tensors ----
            wo_sb = persist.tile([128, 2, D], FP32R, tag="wo")
            ones_col = persist.tile([128, 1], FP32, tag="ones")
            ones_row = persist.tile([1, DH], FP32R, tag="onesr")
            nc.vector.memset(ones_col[:, :], 1.0)
            nc.vector.tensor_copy(
                ones_row[:, :], ones_col[0:1, 0:1].broadcast_to((1, DH))
            )
            # causal mask patterns, generated on-chip (no DMA):
            # mask4[p, j, q] = 1.0 iff 128*j + p <= q
            mask4 = persist.tile([128, 4, S_TILE], FP32, tag="mask4")
            nc.gpsimd.memset(mask4[:, :, :], 1.0)
            for j in range(4):
                nc.gpsimd.affine_select(
                    mask4[:, j, :],
                    mask4[:, j, :],
                    pattern=[[1, S_TILE]],
                    compare_op=mybir.AluOpType.is_ge,
                    fill=0.0,
                    base=-128 * j,
                    channel_multiplier=-1,
                )
            w_sb = {}
            for name in ("q", "k", "v"):
                w_sb[name] = persist.tile(
                    [128, N_DC, GM], BF16, tag=f"w{name}", name=f"w{name}sb"
                )

            qT = {}  # (u, t) -> [128, 512]   2 heads stacked (rows 0-63 / 64-127)
            kT = {}
            vt = {}  # c16 -> [128, HG, 65]   v chunk with ones col per head
            oT = {}  # (u, t) -> [128, 512]
            for t in range(N_ST):
                for u in range(2):
                    qT[(u, t)] = persist.tile(
                        [128, S_TILE], BF16, tag=f"qT{u}{t}", name=f"qT{u}{t}"
                    )
                    kT[(u, t)] = persist.tile(
                        [128, S_TILE], BF16, tag=f"kT{u}{t}", name=f"kT{u}{t}"
                    )
                    oT[(u, t)] = persist.tile(
                        [128, S_TILE], FP32R, tag=f"oT{u}{t}", name=f"oT{u}{t}"
                    )
            for c16 in range(N_KC):
                vt[c16] = persist.tile(
                    [128, HG, DH + 1], BF16, tag=f"v{c16}", name=f"v{c16}"
                )

            xb = {}

            def load_xb(t, c0):
                # two d-chunks per DMA: halves the HWDGE issue cost (625ns
                # per DMA vs 364ns bf16 transfer, so issue rate dominates)
                blk = xb_pool.tile(
                    [128, 2, S_TILE], BF16, tag="xb", name=f"xb{c0}_{t}"
                )
                nc.sync.dma_start(
                    blk[:, :, :],
                    xT[
                        c0 * 128 : (c0 + 2) * 128,
                        t * S_TILE : (t + 1) * S_TILE,
                    ].rearrange("(i p) m -> p i m", i=2),
                    )
                xb[(c0, t)] = blk[:, 0, :]
                xb[(c0 + 1, t)] = blk[:, 1, :]

            # ---- DMA stream order ----
            # tile-0 inputs first (weights in halves interleaved with x blocks
            # so the first k/q matmuls start ~3.5us in), then x tiles 1..3
            # stream ahead of their QKV filler units, then wo (needed late).
            wk_r = wk.rearrange("(c p) m -> p c m", p=128)
            wq_r = wq.rearrange("(c p) m -> p c m", p=128)
            wv_r = wv.rearrange("(c p) m -> p c m", p=128)
            # paced for the chunk-major prologue: per 2-chunk period, the k and
            # q weight chunks land just before the x block pair that uses them
            for p in range(4):
                nc.sync.dma_start(
                    w_sb["k"][:, 2 * p : 2 * p + 2, :], wk_r[:, 2 * p : 2 * p + 2, :]
                )
                if p > 0:
                    nc.sync.dma_start(
                        w_sb["q"][:, 2 * p : 2 * p + 2, :],
                        wq_r[:, 2 * p : 2 * p + 2, :],
                    )
                load_xb(0, 2 * p)
                if p == 0:
                    nc.sync.dma_start(
                        w_sb["q"][:, 0:2, :], wq_r[:, 0:2, :]
                    )
            nc.sync.dma_start(w_sb["v"][:, 0:4, :], wv_r[:, 0:4, :])
            nc.sync.dma_start(w_sb["v"][:, 4:8, :], wv_r[:, 4:8, :])
            for c0 in range(0, N_DC, 2):
                load_xb(1, c0)
            nc.sync.dma_start(wo_sb[:, :, :], wo.rearrange("(u p) d -> p u d", p=128))
            for t in range(2, N_ST):
                for c0 in range(0, N_DC, 2):
                    load_xb(t, c0)

            # ---- emission thunks ----
            # Group PSUM tiles are created lazily by the first unit of each
            # group (cell dict) so mm_pool's buffer rotation follows actual
            # use order, not enqueue order.
            def qk_matmuls(name, u, t, cell, c0, c1):
                if c0 == 0:
                    cell["ps"] = mm_pool.tile(
                        [128, S_TILE], FP32, tag="mm", name=f"ps{name}{u}{t}"
                    )
                ps = cell["ps"]
                for c in range(c0, c1):
                    nc.tensor.matmul(
                        ps[:, :],
                        lhsT=w_sb[name][:, c, u * 128 : (u + 1) * 128],
                        rhs=xb[(c, t)][:, :],
                        start=(c == 0),
                        stop=(c == N_DC - 1),
                    )
                if c1 == N_DC:
                    dst = kT if name == "k" else qT
                    nc.vector.tensor_copy(dst[(u, t)][:, :], ps[:, :])

            def v_matmuls(t, s4, cell, c0, c1):
                c16 = 4 * t + s4
                if c0 == 0:
                    cell["ps"] = mm_pool.tile(
                        [128, GM], FP32, tag="mm", name=f"psv{c16}"
                    )
                ps = cell["ps"]
                for c in range(c0, c1):
                    nc.tensor.matmul(
                        ps[:, :],
                        lhsT=xb[(c, t)][:, s4 * 128 : (s4 + 1) * 128],
                        rhs=w_sb["v"][:, c, :],
                        start=(c == 0),
                        stop=(c == N_DC - 1),
                    )
                if c1 == N_DC:
                    nc.vector.tensor_copy(
                        vt[c16][:, :, 0:DH], ps.rearrange("p (h d) -> p h d", h=HG)
                    )
                    nc.gpsimd.tensor_copy(
                        vt[c16][:, :, DH : DH + 1],
                        ones_col[:, 0:1].broadcast_to((128, HG, 1)),
                    )

            def emit_wo_group(t, dc, on_act=False):
                po = mm_pool.tile([128, S_TILE], FP32, tag="mm", name=f"po{t}{dc}")
                for u in range(2):
                    nc.tensor.matmul(
                        po[:, :],
                        lhsT=wo_sb[:, u, dc * 128 : (dc + 1) * 128],
                        rhs=oT[(u, t)][:, :],
                        start=(u == 0),
                        stop=(u == 1),
                    )
                ob = out_pool.tile([128, S_TILE], BF16, tag="ob")
                if on_act:  # tail: split the copy across ACT+DVE so the PSUM
                    # bank frees at PE pace, not copy pace
                    nc.scalar.copy(ob[:, 0 : S_TILE // 2], po[:, 0 : S_TILE // 2])
                    nc.vector.tensor_copy(ob[:, S_TILE // 2 :], po[:, S_TILE // 2 :])
                else:
                    nc.vector.tensor_copy(ob[:, :], po[:, :])
                nc.sync.dma_start(
                    outT[dc * 128 : (dc + 1) * 128, t * S_TILE : (t + 1) * S_TILE],
                    ob[:, :],
                )

            # filler queue: (tile_tag, cost_cycles, thunk). Attention chunks pump
            # filler units between chunks to keep the PE dense while exp/mask
            # latency elapses. QKV units are tagged with their tile (flushed
            # before that tile's attention); Wo units are tagged 99 (pump/drain
            # only -- they are enqueued once their oT inputs exist).
            fillers = deque()

            def enqueue_qkv_units(t):
                for name in ("k", "q"):
                    for u in range(2):
                        cell = {}
                        for c0 in range(0, N_DC, 2):
                            fillers.append(
                                (
                                    t,
                                    2 * S_TILE,
                                    lambda name=name, u=u, t=t, cell=cell, c0=c0: qk_matmuls(
                                        name, u, t, cell, c0, c0 + 2
                                    ),
                                )
                            )
                for s4 in range(4):
                    cell = {}
                    for c0 in range(0, N_DC, 4):
                        fillers.append(
                            (
                                t,
                                4 * GM,
                                lambda t=t, s4=s4, cell=cell, c0=c0: v_matmuls(
                                    t, s4, cell, c0, c0 + 4
                                ),
                            )
                        )

            # --- emission-time ACT-vs-PE ledger (reset per attention phase).
            # exp work accumulates act_ns; attention matmuls + pumped fillers
            # accumulate pe_ns. Pump fillers whenever ACT is ahead, so the PE
            # stream is diluted exactly where the scalar engine needs time.
            PE_CY = 1.0 / 2.4  # ns per cycle at peak
            SLACK = 900.0  # exp-pipeline fill depth: don't pump before ACT is
            # genuinely ahead of the PE stream in wall-clock terms
            ledger = {"pe": 0.0, "act": -SLACK}

            fins = deque()  # deferred den-broadcast units: popped by pumps at
            # least two chunks into the next phase, when their reciprocal
            # (ACT den copy -> DVE recip) has drained
            phase_chunk = {"c": 0}

            def pump_ledger(max_tag, extra=0.0):
                ledger["act"] += extra
                while ledger["pe"] < ledger["act"] and (fins or fillers):
                    if fins and phase_chunk["c"] >= 2:
                        cost, thunk = fins.popleft()
                    elif fillers:
                        tag, cost, thunk = fillers[0]
                        if tag != 99 and tag > max_tag:
                            return
                        fillers.popleft()
                    else:
                        return
                    thunk()
                    ledger["pe"] += cost * PE_CY

            def flush_tile(t):
                # 99 = wo units (pump/drain only, reserved as late filler)
                while fillers and fillers[0][0] <= t:
                    _, _, thunk = fillers.popleft()
                    thunk()

            # prologue, chunk-major: all four k/q accumulation groups advance
            # per arriving x block (their PSUM lives in sc_pool, idle during
            # the prologue), so the PE has ~850ns of work per ~1.5us DMA chunk
            # instead of ~430ns. v groups become tile-0 fillers.
            sc0 = sc_pool.tile([128, 2 * S_TILE], FP32, tag="sc", name="pro_sc0")
            sc1 = sc_pool.tile([128, 2 * S_TILE], FP32, tag="sc", name="pro_sc1")
            pro = {("k", 0): sc0[:, 0:S_TILE], ("q", 0): sc0[:, S_TILE:],
                   ("k", 1): sc1[:, 0:S_TILE], ("q", 1): sc1[:, S_TILE:]}
            for s4 in range(4):
                cell = {}
                for c0 in range(0, N_DC, 4):
                    fillers.append(
                        (
                            0,
                            4 * GM,
                            lambda s4=s4, cell=cell, c0=c0: v_matmuls(
                                0, s4, cell, c0, c0 + 4
                            ),
                        )
                    )
            for c in range(N_DC):
                for name in ("k", "q"):
                    for u in range(2):
                        nc.tensor.matmul(
                            pro[(name, u)],
                            lhsT=w_sb[name][:, c, u * 128 : (u + 1) * 128],
                            rhs=xb[(c, 0)][:, :],
                            start=(c == 0),
                            stop=(c == N_DC - 1),
                        )
            for u in range(2):  # ACT is idle during the prologue; keep DVE free
                nc.scalar.copy(kT[(u, 0)][:, :], pro[("k", u)])
                nc.scalar.copy(qT[(u, 0)][:, :], pro[("q", u)])
            for t in range(1, N_ST):
                enqueue_qkv_units(t)

            for t in range(N_ST):
                nch = 4 * t + 4
                for hp in range(2):
                    u = hp
                    if t > 0 or hp > 0:
                        flush_tile(t)  # qkv(<=t) must be emitted
                    ledger["pe"] = 0.0
                    ledger["act"] = -SLACK
                    phase_chunk["c"] = 0
                    avs = [
                        av_pool.tile(
                            [DH + 1, S_TILE], FP32, tag="av", name=f"av{t}{hp}{i}"
                        )
                        for i in range(2)
                    ]
                    pending_avs = deque()  # AV trails scores by AV_LAG chunks
                    for c in range(nch):
                        # Diagonal chunks only touch q columns >= 128j
                        # (causal): scores / exp / AV skip the masked prefix.
                        j = c - 4 * t
                        q0 = 128 * j if j >= 0 else 0
                        w = S_TILE - q0
                        sc = sc_pool.tile([128, 2 * S_TILE], FP32, tag="sc")
                        for i in range(2):  # head parity: rows 0-63 / 64-127
                            bp = 64 * i
                            nc.tensor.matmul(
                                sc[:, i * S_TILE + q0 : (i + 1) * S_TILE],
                                lhsT=kT[(u, c // 4)][
                                    bp : bp + DH, (c % 4) * 128 : (c % 4 + 1) * 128
                                ],
                                rhs=qT[(u, t)][bp : bp + DH, q0:],
                                start=True,
                                stop=True,
                            )
                        ledger["pe"] += 2 * w * PE_CY
                        ex = exp_pool.tile([128, 2 * S_TILE], BF16, tag="ex")
                        exv = ex.rearrange("p (i n) -> p i n", i=2)[:, :, q0:]
                        scv = sc.rearrange("p (i n) -> p i n", i=2)[:, :, q0:]
                        nc.scalar.activation(
                            exv, scv, mybir.ActivationFunctionType.Exp, scale=0.125
                        )
                        ledger["act"] += 2 * w * 0.833 + 242
                        if j >= 0:
                            # zero the causal triangle (mask is 0/1, exact).
                            # Only the band [q0, 128j+128) needs masking; one
                            # op covers both head slots via a broadcast mask.
                            bhi = 128 * j + 128
                            exb = ex.rearrange("p (i n) -> p i n", i=2)[
                                :, :, q0:bhi
                            ]
                            nc.gpsimd.tensor_mul(
                                exb,
                                exb,
                                mask4[:, j : j + 1, q0:bhi].broadcast_to(
                                    (128, 2, bhi - q0)
                                ),
                            )

                        def emit_av(cc, exx, qq0):
                            for i in range(2):
                                nc.tensor.matmul(
                                    avs[i][:, qq0:],
                                    lhsT=vt[cc][:, 2 * hp + i, :],
                                    rhs=exx[:, i * S_TILE + qq0 : (i + 1) * S_TILE],
                                    start=(cc == 0),
                                    stop=(cc == nch - 1),
                                )
                            ledger["pe"] += 2 * (S_TILE - qq0) * PE_CY

                        pending_avs.append((c, ex, q0))
                        phase_chunk["c"] = c
                        pump_ledger(t + 1)
                        if len(pending_avs) > AV_LAG:
                            emit_av(*pending_avs.popleft())
                    if t == 0 and hp == 0:
                        flush_tile(0)  # v(t0) needed by the AV drain below
                    while pending_avs:
                        emit_av(*pending_avs.popleft())
                        # the exp tail is still draining on ACT in wall-clock
                        # terms; keep filler between the drain AVs
                        pump_ledger(t + 1, extra=800)
                    # softmax denominators: rec_i = 1/den_i broadcast down the
                    # 64 partitions via a K=1 ones matmul (per head slot).
                    # The DVE chain (den copy -> reciprocal -> fp32r cast) has
                    # ~2us latency before the bc matmul can run: pump filler
                    # through it via the ledger's extra term.
                    # softmax denominators: only den copy (ACT) + reciprocal
                    # (DVE) run here; the bc broadcast matmul + oT muls are
                    # deferred as a front-of-queue filler unit so the PE never
                    # parks on the reciprocal chain -- the next phase pumps the
                    # unit 1-2 chunks in, when the reciprocal is long done.
                    recs = []
                    for i in range(2):
                        den = small_pool.tile([1, S_TILE], FP32, tag="den")
                        nc.scalar.copy(den[:, :], avs[i][DH : DH + 1, :])
                        rec32 = small_pool.tile([1, S_TILE], FP32, tag="rec32")
                        nc.vector.reciprocal_approx_fast(rec32[:, :], den[:, :])
                        recs.append(rec32)

                    final = t == N_ST - 1 and hp == 1

                    def emit_fin(avs=avs, recs=recs, u=u, t=t, hp=hp, final=final):
                        for i in range(2):
                            rec = small_pool.tile([1, S_TILE], FP32R, tag="rec")
                            nc.vector.tensor_copy(rec[:, :], recs[i][:, :])
                            bc = mm_pool.tile(
                                [DH, S_TILE], FP32, tag="mm", name=f"bc{t}{hp}{i}"
                            )
                            nc.tensor.matmul(
                                bc[:, :],
                                lhsT=ones_row[:, :],
                                rhs=rec[:, :],
                                start=True,
                                stop=True,
                            )
                            bc_sb = small_pool.tile([DH, S_TILE], FP32, tag="bcsb")
                            if final:  # ACT is idle at the kernel tail; keep
                                nc.scalar.copy(bc_sb[:, :], bc[:, :])  # DVE for
                            else:  # the muls so the chain cross-pipelines
                                nc.vector.tensor_copy(bc_sb[:, :], bc[:, :])
                            nc.vector.tensor_mul(
                                oT[(u, t)][64 * i : 64 * i + DH, :],
                                avs[i][0:DH, :],
                                bc_sb[:, :],
                            )

                    if t == N_ST - 1 and hp == 1:
                        emit_fin()  # no later phase to pump it
                    else:
                        fins.append((2 * S_TILE, emit_fin))
                # Wo for this tile becomes filler work for later attention
                # (the last tile's Wo is the kernel tail, emitted below)
                if t < N_ST - 1:
                    for dc in range(N_DC):
                        fillers.append(
                            (99, 2 * S_TILE, lambda t=t, dc=dc: emit_wo_group(t, dc))
                        )
            while fillers:
                fillers.popleft()[2]()
            while fins:
                fins.popleft()[1]()
            # kernel tail: the last tile's 8 Wo groups across 8 borrowed PSUM
            # slots (scores/av pools are idle now). All u=0 matmuls go first --
            # they only need oT(0,t3), which is ready long before the final
            # den-broadcast chain produces oT(1,t3) -- so the PE crunches
            # 3.4us of u0 work while that chain drains. Then the u=1 matmuls
            # close each accumulation, with ob copies split ACT/DVE.
            tail_a = sc_pool.tile([128, 2 * S_TILE], FP32, tag="sc", name="tail_a")
            tail_b = sc_pool.tile([128, 2 * S_TILE], FP32, tag="sc", name="tail_b")
            slots = [
                tail_a[:, 0:S_TILE],
                tail_a[:, S_TILE:],
                tail_b[:, 0:S_TILE],
                tail_b[:, S_TILE:],
                mm_pool.tile([128, S_TILE], FP32, tag="mm", name="tail_m0")[:, :],
                mm_pool.tile([128, S_TILE], FP32, tag="mm", name="tail_m1")[:, :],
                av_pool.tile([128, S_TILE], FP32, tag="av", name="tail_v0")[:, :],
                av_pool.tile([128, S_TILE], FP32, tag="av", name="tail_v1")[:, :],
            ]
            tl = N_ST - 1
            for u in range(2):
                for dc in range(N_DC):
                    nc.tensor.matmul(
                        slots[dc],
                        lhsT=wo_sb[:, u, dc * 128 : (dc + 1) * 128],
                        rhs=oT[(u, tl)][:, :],
                        start=(u == 0),
                        stop=(u == 1),
                    )
                    if u == 1 and dc % 2 == 1:
                        # same-tile writes serialize, so both halves of a pair
                        # go to ONE engine; pairs alternate ACT/DVE so two
                        # pairs' copies run concurrently. One DMA per pair
                        # (HWDGE issue cost dominates the bf16 transfer).
                        ob = out_pool.tile(
                            [128, 2, S_TILE], BF16, tag="ob", name=f"ob{dc}"
                        )
                        nc.scalar.copy(ob[:, 0, :], slots[dc - 1])
                        nc.vector.tensor_copy(ob[:, 1, :], slots[dc])
                        nc.sync.dma_start(
                            outT[
                                (dc - 1) * 128 : (dc + 1) * 128,
                                tl * S_TILE : (tl + 1) * S_TILE,
                            ].rearrange("(i p) m -> p i m", i=2),
                            ob[:, :, :],
                        )
    nc.compile()
    return nc


_NC_CACHE = None


def _get_program():
    global _NC_CACHE
    if _NC_CACHE is None:
        _NC_CACHE = build_program()
    return _NC_CACHE


def _make_in_maps(x, Wq, Wk, Wv, Wo):
    import ml_dtypes

    bf16 = ml_dtypes.bfloat16
    xTs = [np.ascontiguousarray(x[b].T).astype(bf16) for b in range(B)]
    in_maps = []
    for core in range(N_CORES):
        b, g = divmod(core, HG)
        r0, r1 = g * GM, (g + 1) * GM
        in_maps.append(
            {
                "xT": xTs[b],
                "wq": np.ascontiguousarray(Wq[r0:r1, :].T).astype(bf16),
                "wk": np.ascontiguousarray(Wk[r0:r1, :].T).astype(bf16),
                "wv": np.ascontiguousarray(Wv[r0:r1, :].T).astype(bf16),
                "wo": np.ascontiguousarray(Wo[:, r0:r1].T),
            }
        )
    return in_maps


def kernel(x, Wq, Wk, Wv, Wo, **_unused):
    x = np.asarray(x, dtype=np.float32)
    Wq = np.asarray(Wq, dtype=np.float32)
    Wk = np.asarray(Wk, dtype=np.float32)
    Wv = np.asarray(Wv, dtype=np.float32)
    Wo = np.asarray(Wo, dtype=np.float32)

    nc = _get_program()
    in_maps = _make_in_maps(x, Wq, Wk, Wv, Wo)
    res = run_bass_kernel_spmd(nc, in_maps, core_ids=list(range(N_CORES)))
    out = np.zeros((B, S, D), dtype=np.float64)
    for core in range(N_CORES):
        b = core // HG
        out[b] += res.results[core]["outT"].T.astype(np.float64)
    return out.astype(np.float32)
